# revision 25
# baseline (speedup 1.0000x reference)
"""Trainium2 Bass kernel for nn_MultiHeadAttention_67379446939752.

Per-token multi-head attention:
  Q = q @ Wq.T + bq ; K,V likewise        [B,S,D] -> [B,S,H,HD]
  score[t,h,g] = sum_d Q[t,h,d] K[t,g,d]  (per-token HxH gram, no seq mixing)
  attn[t] = softmax(score[t]) @ V[t]      -> [B,S,D]
  out = attn @ Wo.T + bo
with B,S,D,H = 4,4096,2048,16.

Strategy: data-parallel over the 16384 tokens across 8 NeuronCores (2048
tokens/core).  Everything moves in fp16: activations arrive pre-transposed
[D,T] fp16, and the four 2048x2048 weights are sharded 8-ways by contraction
rows (4 MB/core) and AllGathered on device, so per-call host->device traffic
is ~288 MB instead of ~1 GB (weights were previously replicated x8 in fp32).
Matmuls run fp16 x fp16 -> fp32 PSUM at full PE rate.  The per-token 16x16
attention is computed 8 tokens at a time as one 128x128x128 matmul whose
cross-token blocks are pushed to -1024 in PSUM by a rank-8 mask matmul;
exp() then zeroes them exactly, so the block-diagonal softmax needs no
masking pass.  Output returns as fp16 and is upcast on host.
"""
import sys
sys.path.insert(0, "/opt/trn_rl_repo")
import numpy as np
import concourse.bass as bass
import concourse.mybir as mybir
import concourse.bacc as bacc
import concourse.tile as tile
from concourse.bass_utils import run_bass_kernel_spmd

B, S, D, H, HD = 4, 4096, 2048, 16, 128
NCORES = 8
T_FULL = B * S
F32, F16 = mybir.dt.float32, mybir.dt.float16
KT = D // 128             # contraction tiles
RSH = D // NCORES         # weight rows per rank shard (256)
SHIFT = 25.0              # constant softmax shift (softmax-invariant)
NEG = 1024.0              # additive mask magnitude for cross-token blocks
TA = 256                  # token chunk (phase A/B/C share this granularity)
Exp = mybir.ActivationFunctionType.Exp


def mask_consts():
    # u8[r,(t,h)] = 1 if t==r ; v8[r,(t',g)] = -NEG*(1 - (t'==r))
    u = np.zeros((8, 128), np.float32)
    for r in range(8):
        u[r, r * 16:(r + 1) * 16] = 1.0
    v = np.full((8, 128), -NEG, np.float32)
    for r in range(8):
        v[r, r * 16:(r + 1) * 16] = 0.0
    return u, v


def build(T, repeat=1, trace_sim=False):
    TAe = min(TA, T)
    NCH = T // TAe           # chunks
    NBK = TAe // 8           # 8-token blocks per chunk
    nc = bacc.Bacc(None, target_bir_lowering=False)
    qT = nc.dram_tensor("qT", [D, T], F16, kind="ExternalInput")
    kT = nc.dram_tensor("kT", [D, T], F16, kind="ExternalInput")
    vT = nc.dram_tensor("vT", [D, T], F16, kind="ExternalInput")
    # rank shard: rows [c*256:(c+1)*256] of WqT,WkT,WvT,WoT stacked -> [1024,D]
    Wsh = nc.dram_tensor("Wsh", [4 * RSH, D], F16, kind="ExternalInput")
    bqT = nc.dram_tensor("bqT", [128, H], F32, kind="ExternalInput")
    bkT = nc.dram_tensor("bkT", [128, H], F32, kind="ExternalInput")
    bvT = nc.dram_tensor("bvT", [128, H], F32, kind="ExternalInput")
    bo_row = nc.dram_tensor("bo_row", [1, D], F16, kind="ExternalInput")
    ones_row = nc.dram_tensor("ones_row", [1, 128], F16, kind="ExternalInput")
    out_d = nc.dram_tensor("out", [T, D], F16, kind="ExternalOutput")

    u8_np, v8_np = mask_consts()
    u8_d = nc.inline_tensor(u8_np.astype(np.float16), "u8c")
    v8_d = nc.inline_tensor(v8_np.astype(np.float16), "v8c")
    id_d = nc.inline_tensor(np.eye(128, dtype=np.float16), "id128")

    with tile.TileContext(nc, trace_sim=trace_sim) as tc:
        with (
            tc.tile_pool(name="dram", bufs=1, space="DRAM") as dpool,
            tc.tile_pool(name="const", bufs=1) as cpool,
        ):
            # ---- weight AllGathers, split so Wq lands first:
            #   AG1: rank's Wq rows [256,D] -> [2048,D]        (gates phase A start)
            #   AG2: rank's Wk|Wv|Wo rows [768,D] -> [6144,D]  (hides behind Q proj)
            agq_in = dpool.tile([RSH, D], F16, tag="agq_in", name="agq_in")
            agq_out = dpool.tile([NCORES * RSH, D], F16, tag="agq_out",
                                 name="agq_out", addr_space="Shared")
            ag_in = dpool.tile([3 * RSH, D], F16, tag="ag_in", name="ag_in")
            ag_out = dpool.tile([NCORES * 3 * RSH, D], F16, tag="ag_out",
                                name="ag_out", addr_space="Shared")
            nc.gpsimd.dma_start(agq_in[:], Wsh[0:RSH, :])
            nc.gpsimd.collective_compute(
                "AllGather", mybir.AluOpType.bypass,
                replica_groups=[list(range(NCORES))],
                ins=[agq_in.opt()], outs=[agq_out.opt()],
            )
            nc.gpsimd.dma_start(ag_in[:], Wsh[RSH:4 * RSH, :])
            nc.gpsimd.collective_compute(
                "AllGather", mybir.AluOpType.bypass,
                replica_groups=[list(range(NCORES))],
                ins=[ag_in.opt()], outs=[ag_out.opt()],
            )
            # views [128p, a, D]:
            #  AGqv: gathered Wq row = a*128+p, a = rank*2 + i          (a in 0..16)
            #  AGv: gathered Wk/Wv/Wo row: a = rank*6 + blk*2 + i, blk k=0 v=1 o=2
            AGqv = agq_out[:].rearrange("(a p) j -> p a j", p=128)
            AGv = ag_out[:].rearrange("(a p) j -> p a j", p=128)

            # per-chunk spill tiles (fine-grained cross-phase deps)
            QT_ds = [dpool.tile([128, TAe * H], F16, tag=f"QTd{i}", name=f"QTd{i}") for i in range(NCH)]
            KT_ds = [dpool.tile([128, TAe * H], F16, tag=f"KTd{i}", name=f"KTd{i}") for i in range(NCH)]
            VT_ds = [dpool.tile([128, TAe * H], F16, tag=f"VTd{i}", name=f"VTd{i}") for i in range(NCH)]
            ATT_ds = [dpool.tile([D, TAe], F16, tag=f"ATTd{i}", name=f"ATTd{i}") for i in range(NCH)]

            u8 = cpool.tile([8, 128], F16, tag="u8")
            v8 = cpool.tile([8, 128], F16, tag="v8")
            ident = cpool.tile([128, 128], F16, tag="ident")
            nc.sync.dma_start(u8[:], u8_d[:])
            nc.sync.dma_start(v8[:], v8_d[:])
            nc.sync.dma_start(ident[:], id_d[:])
            biasq = cpool.tile([128, H], F32, tag="bq")
            biask = cpool.tile([128, H], F32, tag="bk")
            biasv = cpool.tile([128, H], F32, tag="bvt")
            bor = cpool.tile([1, D], F16, tag="bo")
            onesr = cpool.tile([1, 128], F16, tag="ones")
            nc.sync.dma_start(biasq[:], bqT[:])
            nc.sync.dma_start(biask[:], bkT[:])
            nc.sync.dma_start(biasv[:], bvT[:])
            nc.sync.dma_start(bor[:], bo_row[:])
            nc.sync.dma_start(onesr[:], ones_row[:])
            shiftc = cpool.tile([128, 1], F32, tag="shiftc")
            nc.vector.memset(shiftc[:], -SHIFT)

            def _load_w(pool, which, tag):
                # quarter q covers contraction tiles it=4q..4q+4, i.e. WT rows
                # 512q..512q+512 = rank 2q (i=0,1) then rank 2q+1 (i=0,1)
                parts = []
                for q in range(4):
                    wq = pool.tile([128, 4, D], F16, tag=f"{tag}{q}", name=f"{tag}{q}")
                    if which == "q":
                        nc.sync.dma_start(wq[:], AGqv[:, 4 * q:4 * q + 4, :])
                    else:
                        blk = {"k": 0, "v": 1, "o": 2}[which]
                        a0 = 12 * q + 2 * blk
                        nc.sync.dma_start(wq[:, 0:2, :], AGv[:, a0:a0 + 2, :])
                        nc.sync.dma_start(wq[:, 2:4, :], AGv[:, a0 + 6:a0 + 8, :])
                    parts.append(wq)
                return parts

            def _phases():
                # ---------------- Phase A: projections ----------------
                with (
                    tc.tile_pool(name="wt", bufs=1) as wpool,
                    tc.tile_pool(name="xs", bufs=2) as xpool,
                    tc.tile_pool(name="psA", bufs=8, space="PSUM") as psA,
                    tc.tile_pool(name="stA", bufs=1) as stA,
                ):
                    for xin, which, bias, spills in (
                        (qT, "q", biasq, QT_ds),
                        (kT, "k", biask, KT_ds),
                        (vT, "v", biasv, VT_ds),
                    ):
                        xs0 = xpool.tile([128, KT, TAe], F16, tag="xs", name="xs0")
                        nc.sync.dma_start(
                            xs0[:], xin[:, 0:TAe].rearrange("(it p) t -> p it t", p=128))
                        wt = _load_w(wpool, which, "wt")
                        for c in range(NCH):
                            if c == 0:
                                xs = xs0
                            else:
                                xs = xpool.tile([128, KT, TAe], F16, tag="xs")
                                nc.sync.dma_start(
                                    xs[:], xin[:, c * TAe:(c + 1) * TAe].rearrange(
                                        "(it p) t -> p it t", p=128))
                            stg = stA.tile([128, TAe, H], F16, tag="stA")
                            for jh in range(2):
                                pss = [psA.tile([128, TAe], F32, tag="psA",
                                                name=f"psA{jh}_{j}") for j in range(8)]
                                for q in range(4):
                                    for jl in range(8):
                                        jt = jh * 8 + jl
                                        for kl in range(4):
                                            ki = q * 4 + kl
                                            nc.tensor.matmul(
                                                pss[jl][:], wt[q][:, kl, jt * 128:(jt + 1) * 128],
                                                xs[:, ki, :], start=(ki == 0), stop=(ki == KT - 1))
                                for jl in range(8):
                                    jt = jh * 8 + jl
                                    nc.any.tensor_scalar_add(stg[:, :, jt], pss[jl][:],
                                                             bias[:, jt:jt + 1])
                            nc.sync.dma_start(
                                spills[c][:], stg[:].rearrange("p t h -> p (t h)"))

                # ---------------- Phase B (with Wo q0 prefetch) + C ----------------
                with tc.tile_pool(name="wo", bufs=1) as wopool:
                  wo0 = wopool.tile([128, 4, D], F16, tag="wo0", name="wo0")
                  nc.sync.dma_start(wo0[:, 0:2, :], AGv[:, 4:6, :])
                  nc.sync.dma_start(wo0[:, 2:4, :], AGv[:, 10:12, :])
                  with (
                      tc.tile_pool(name="qk", bufs=2) as qkpool,
                      tc.tile_pool(name="vb", bufs=2) as vpool,
                      tc.tile_pool(name="attc", bufs=2) as apool,
                      tc.tile_pool(name="eb", bufs=6) as epool,
                      tc.tile_pool(name="zb", bufs=8) as zpool,
                      tc.tile_pool(name="psS", bufs=2, space="PSUM") as psS,
                      tc.tile_pool(name="psT", bufs=2, space="PSUM") as psT,
                      tc.tile_pool(name="psV", bufs=2, space="PSUM") as psV,
                      tc.tile_pool(name="psA2", bufs=2, space="PSUM") as psA2,
                  ):
                      for c in range(NCH):
                          QTs = qkpool.tile([128, TAe, H], F16, tag="QTs")
                          KTs = qkpool.tile([128, TAe, H], F16, tag="KTs")
                          nc.gpsimd.dma_start(
                              QTs[:], QT_ds[c][:].rearrange("p (t h) -> p t h", h=H))
                          nc.gpsimd.dma_start(
                              KTs[:], KT_ds[c][:].rearrange("p (t h) -> p t h", h=H))
                          VTs = vpool.tile([128, TAe, H], F16, tag="VTs")
                          nc.sync.dma_start(
                              VTs[:], VT_ds[c][:].rearrange("p (t h) -> p t h", h=H))
                          ATTc = apool.tile([128, H, TAe], F16, tag="ATTc")
                          for bk in range(NBK):
                              sl = slice(bk * 8, (bk + 1) * 8)
                              w0 = (bk // 2) * 2            # even-aligned 2-block window
                              off = (bk % 2) * 128          # valid column offset
                              slw = slice(w0 * 8, (w0 + 2) * 8)
                              ps_b = psS.tile([128, 256], F32, tag="ps_s")
                              nc.tensor.matmul(
                                  ps_b[:],
                                  QTs[:, sl, :].rearrange("p t h -> p (t h)"),
                                  KTs[:, slw, :].rearrange("p t h -> p (t h)"),
                                  start=True, stop=False, skip_group_check=True)
                              nc.tensor.matmul(ps_b[:, off:off + 128], u8[:], v8[:],
                                               start=False, stop=True, skip_group_check=True)
                              E = epool.tile([128, 128], F32, tag="E")
                              Z = zpool.tile([128, 1], F32, tag="Z")
                              nc.scalar.activation(E[:], ps_b[:, off:off + 128], Exp,
                                                   bias=shiftc[:], accum_out=Z[:])
                              R = zpool.tile([128, 1], F32, tag="R")
                              nc.vector.reciprocal(R[:], Z[:])
                              Wb = epool.tile([128, 128], F16, tag="Wb")
                              nc.vector.tensor_scalar_mul(Wb[:], E[:], R[:])
                              ps_t = psT.tile([128, 128], F16, tag="ps_t")
                              nc.tensor.transpose(ps_t[:], Wb[:], ident[:])
                              WTs = epool.tile([128, 128], F16, tag="WTs")
                              nc.any.tensor_copy(WTs[:], ps_t[:])
                              ps_v = psV.tile([128, 128], F16, tag="ps_v")
                              nc.tensor.transpose(
                                  ps_v[:], VTs[:, sl, :].rearrange("p t h -> p (t h)"),
                                  ident[:])
                              Vb = epool.tile([128, 128], F16, tag="Vb")
                              nc.any.tensor_copy(Vb[:], ps_v[:])
                              ps_a = psA2.tile([128, 128], F32, tag="ps_a")
                              nc.tensor.matmul(ps_a[:], Vb[:], WTs[:],
                                               start=True, stop=True)
                              nc.any.tensor_copy(
                                  ATTc[:, :, bk * 8:(bk + 1) * 8].rearrange("p h t -> p t h"),
                                  ps_a[:].rearrange("p (t h) -> p t h", t=8))
                          nc.sync.dma_start(
                              ATT_ds[c][:].rearrange("(h p) t -> p h t", p=128), ATTc[:])

                  # ---------------- Phase C: output projection ----------------
                  with (
                      tc.tile_pool(name="ca", bufs=2) as capool,
                      tc.tile_pool(name="psC", bufs=8, space="PSUM") as psC,
                      tc.tile_pool(name="stC", bufs=4) as stC,
                  ):
                      ATTs0 = capool.tile([128, H, TAe], F16, tag="ATTs", name="ATTs0")
                      nc.sync.dma_start(
                          ATTs0[:], ATT_ds[0][:].rearrange("(h p) t -> p h t", p=128))
                      wo = [wo0]
                      for q in range(1, 4):
                          wq = wopool.tile([128, 4, D], F16, tag=f"wo{q}", name=f"wo{q}")
                          a0 = 12 * q + 4
                          nc.sync.dma_start(wq[:, 0:2, :], AGv[:, a0:a0 + 2, :])
                          nc.sync.dma_start(wq[:, 2:4, :], AGv[:, a0 + 6:a0 + 8, :])
                          wo.append(wq)
                      for cc in range(NCH):
                          if cc == 0:
                              ATTs = ATTs0
                          else:
                              ATTs = capool.tile([128, H, TAe], F16, tag="ATTs")
                              nc.sync.dma_start(
                                  ATTs[:], ATT_ds[cc][:].rearrange("(h p) t -> p h t", p=128))
                          tjs = [(tt, jc) for tt in range(TAe // 128) for jc in range(D // 512)]
                          pss = [psC.tile([128, 512], F32, tag="psC", name=f"psC{i}")
                                 for i in range(len(tjs))]
                          for hq in range(4):
                              for i, (tt, jc) in enumerate(tjs):
                                  for hl in range(4):
                                      h = hq * 4 + hl
                                      nc.tensor.matmul(
                                          pss[i][:], ATTs[:, h, tt * 128:(tt + 1) * 128],
                                          wo[hq][:, hl, jc * 512:(jc + 1) * 512],
                                          start=(h == 0), stop=False)
                          for i, (tt, jc) in enumerate(tjs):
                              nc.tensor.matmul(pss[i][:], onesr[:], bor[:, jc * 512:(jc + 1) * 512],
                                               start=False, stop=True)
                              st = stC.tile([128, 512], F16, tag="stC")
                              nc.any.tensor_copy(st[:], pss[i][:])
                              nc.sync.dma_start(
                                  out_d[cc * TAe + tt * 128: cc * TAe + (tt + 1) * 128,
                                        jc * 512:(jc + 1) * 512], st[:])

            for _rep in range(repeat):
                _phases()
    nc.compile()
    return nc


_cache = {}


def get_nc(T):
    if T not in _cache:
        _cache[T] = build(T)
    return _cache[T]


def make_in_maps(q, k, v, Wq, bq, Wk, bk, Wv, bv, Wo, bo, ncores=NCORES, T=None):
    f, h = np.float32, np.float16
    q = np.asarray(q, f).reshape(-1, D)
    k = np.asarray(k, f).reshape(-1, D)
    v = np.asarray(v, f).reshape(-1, D)
    if T is None:
        T = q.shape[0] // ncores
    WqT = np.asarray(Wq, f).T
    WkT = np.asarray(Wk, f).T
    WvT = np.asarray(Wv, f).T
    WoT = np.asarray(Wo, f).T
    bqT = np.ascontiguousarray(np.asarray(bq, f).reshape(H, 128).T)
    bkT = np.ascontiguousarray(np.asarray(bk, f).reshape(H, 128).T)
    bvTc = np.ascontiguousarray(np.asarray(bv, f).reshape(H, 128).T)
    bor = np.asarray(bo, f).reshape(1, D).astype(h)
    maps = []
    for c in range(ncores):
        sl = slice(c * T, (c + 1) * T)
        rs = slice(c * RSH, (c + 1) * RSH)
        maps.append({
            "qT": q[sl].T.astype(h),
            "kT": k[sl].T.astype(h),
            "vT": v[sl].T.astype(h),
            "Wsh": np.vstack([WqT[rs], WkT[rs], WvT[rs], WoT[rs]]).astype(h),
            "bqT": bqT, "bkT": bkT, "bvT": bvTc, "bo_row": bor,
            "ones_row": np.ones((1, 128), h),
        })
    return maps, T


def kernel(q, k, v, Wq, bq, Wk, bk, Wv, bv, Wo, bo):
    maps, T = make_in_maps(q, k, v, Wq, bq, Wk, bk, Wv, bv, Wo, bo)
    nc = get_nc(T)
    res = run_bass_kernel_spmd(nc, maps, list(range(NCORES)))
    out = np.concatenate([np.asarray(r["out"]) for r in res.results], axis=0)
    return out.reshape(B, S, D).astype(np.float32)


# revision 44
# speedup vs baseline: 1.1991x; 1.1991x over previous
"""Trainium2 Bass kernel for nn_MultiHeadAttention_67379446939752.

Per-token multi-head attention:
  Q = q @ Wq.T + bq ; K,V likewise        [B,S,D] -> [B,S,H,HD]
  score[t,h,g] = sum_d Q[t,h,d] K[t,g,d]  (per-token HxH gram, no seq mixing)
  attn[t] = softmax(score[t]) @ V[t]      -> [B,S,D]
  out = attn @ Wo.T + bo
with B,S,D,H = 4,4096,2048,16.

Strategy: data-parallel over the 16384 tokens across 8 NeuronCores (2048
tokens/core).  Everything moves in fp16: activations arrive pre-transposed
[D,T] fp16, and the four 2048x2048 weights are sharded 8-ways by contraction
rows (4 MB/core) and AllGathered on device (Wq gathered first so phase A
starts immediately; both collectives sit on the in-order gpsimd queue to pin
their order).  Per-call host->device traffic is ~235 MB + 67 MB out instead
of ~896 MB + 128 MB (weights were previously replicated x8 in fp32).
Matmuls run fp16 x fp16 -> fp32 PSUM at full PE rate.  The per-token 16x16
attention is computed 8 tokens at a time as one 128x128x128 matmul whose
cross-token blocks are pushed to -1024 in PSUM by a rank-8 mask matmul;
exp() then zeroes them exactly, so the block-diagonal softmax needs no
masking pass.  Output returns as fp16 and is upcast on host.
"""
import sys
sys.path.insert(0, "/opt/trn_rl_repo")
import numpy as np
import concourse.bass as bass
import concourse.mybir as mybir
import concourse.bacc as bacc
import concourse.tile as tile
from concourse.bass_utils import run_bass_kernel_spmd

B, S, D, H, HD = 4, 4096, 2048, 16, 128
NCORES = 8
T_FULL = B * S
F32, F16 = mybir.dt.float32, mybir.dt.float16
KT = D // 128             # contraction tiles
RSH = D // NCORES         # weight rows per rank shard (256)
SHIFT = 25.0              # constant softmax shift (softmax-invariant)
NEG = 1024.0              # additive mask magnitude for cross-token blocks
TA = 256                  # token chunk (phase A/B/C share this granularity)
Exp = mybir.ActivationFunctionType.Exp


def mask_consts():
    # u8[r,(t,h)] = 1 if t==r ; v8[r,(t',g)] = -NEG*(1 - (t'==r))
    u = np.zeros((8, 128), np.float32)
    for r in range(8):
        u[r, r * 16:(r + 1) * 16] = 1.0
    v = np.full((8, 128), -NEG, np.float32)
    for r in range(8):
        v[r, r * 16:(r + 1) * 16] = 0.0
    return u, v


def build(T, repeat=1, trace_sim=False):
    TAe = min(TA, T)
    NCH = T // TAe           # chunks
    NBK = TAe // 8           # 8-token blocks per chunk
    nc = bacc.Bacc(None, target_bir_lowering=False)
    qT = nc.dram_tensor("qT", [D, T], F16, kind="ExternalInput")
    kT = nc.dram_tensor("kT", [D, T], F16, kind="ExternalInput")
    vT = nc.dram_tensor("vT", [D, T], F16, kind="ExternalInput")
    # Wq replicated (gates phase A start: no collective wait before first matmul)
    WqTf = nc.dram_tensor("WqTf", [D, D], F16, kind="ExternalInput")
    # rank shard: rows [c*256:(c+1)*256] of WkT,WvT,WoT stacked -> [768,D]
    Wsh = nc.dram_tensor("Wsh", [3 * RSH, D], F16, kind="ExternalInput")
    bqT = nc.dram_tensor("bqT", [128, H], F32, kind="ExternalInput")
    bkT = nc.dram_tensor("bkT", [128, H], F32, kind="ExternalInput")
    bvT = nc.dram_tensor("bvT", [128, H], F32, kind="ExternalInput")
    bo_row = nc.dram_tensor("bo_row", [1, D], F16, kind="ExternalInput")
    ones_row = nc.dram_tensor("ones_row", [1, 128], F16, kind="ExternalInput")
    out_d = nc.dram_tensor("out", [T, D], F16, kind="ExternalOutput")

    u8_np, v8_np = mask_consts()
    u8_d = nc.inline_tensor(u8_np.astype(np.float16), "u8c")
    v8_d = nc.inline_tensor(v8_np.astype(np.float16), "v8c")
    id_d = nc.inline_tensor(np.eye(128, dtype=np.float16), "id128")

    with tile.TileContext(nc, trace_sim=trace_sim) as tc:
        with (
            tc.tile_pool(name="dram", bufs=1, space="DRAM") as dpool,
            tc.tile_pool(name="const", bufs=1) as cpool,
        ):
            # ---- weight AllGathers for Wk then Wv|Wo (hide behind the Q/K
            # projections; Q reads the replicated WqTf input directly)
            agk_in = dpool.tile([RSH, D], F16, tag="agk_in", name="agk_in")
            agk_out = dpool.tile([NCORES * RSH, D], F16, tag="agk_out",
                                 name="agk_out", addr_space="Shared")
            ag_in = dpool.tile([2 * RSH, D], F16, tag="ag_in", name="ag_in")
            ag_out = dpool.tile([NCORES * 2 * RSH, D], F16, tag="ag_out",
                                name="ag_out", addr_space="Shared")
            nc.gpsimd.dma_start(agk_in[:], Wsh[0:RSH, :])
            nc.gpsimd.collective_compute(
                "AllGather", mybir.AluOpType.bypass,
                replica_groups=[list(range(NCORES))],
                ins=[agk_in.opt()], outs=[agk_out.opt()],
            )
            nc.gpsimd.dma_start(ag_in[:], Wsh[RSH:3 * RSH, :])
            nc.gpsimd.collective_compute(
                "AllGather", mybir.AluOpType.bypass,
                replica_groups=[list(range(NCORES))],
                ins=[ag_in.opt()], outs=[ag_out.opt()],
            )
            # views [128p, a, D]:
            #  Wqv:  replicated Wq row = a*128+p                    (a in 0..16)
            #  AGkv: gathered Wk row = a*128+p, a = rank*2 + i      (a in 0..16)
            #  AGv:  gathered Wv/Wo row: a = rank*4 + blk*2 + i, blk v=0 o=1
            Wqv = WqTf.ap().rearrange("(a p) j -> p a j", p=128)
            AGkv = agk_out[:].rearrange("(a p) j -> p a j", p=128)
            AGv = ag_out[:].rearrange("(a p) j -> p a j", p=128)

            # per-chunk spill tiles (fine-grained cross-phase deps)
            QT_ds = [dpool.tile([128, TAe * H], F16, tag=f"QTd{i}", name=f"QTd{i}") for i in range(NCH)]
            KT_ds = [dpool.tile([128, TAe * H], F16, tag=f"KTd{i}", name=f"KTd{i}") for i in range(NCH)]
            VT_ds = [dpool.tile([128, TAe * H], F16, tag=f"VTd{i}", name=f"VTd{i}") for i in range(NCH)]
            ATT_ds = [dpool.tile([D, TAe], F16, tag=f"ATTd{i}", name=f"ATTd{i}") for i in range(NCH)]

            # const loads on the DVE queue: keeps the SP queue free for the
            # first xs/wt loads that gate the first matmul
            u8 = cpool.tile([8, 128], F16, tag="u8")
            v8 = cpool.tile([8, 128], F16, tag="v8")
            ident = cpool.tile([128, 128], F16, tag="ident")
            nc.scalar.dma_start(u8[:], u8_d[:])
            nc.scalar.dma_start(v8[:], v8_d[:])
            nc.scalar.dma_start(ident[:], id_d[:])
            biasq = cpool.tile([128, H], F32, tag="bq")
            biask = cpool.tile([128, H], F32, tag="bk")
            biasv = cpool.tile([128, H], F32, tag="bvt")
            bor = cpool.tile([1, D], F16, tag="bo")
            onesr = cpool.tile([1, 128], F16, tag="ones")
            nc.scalar.dma_start(biasq[:], bqT[:])
            nc.scalar.dma_start(biask[:], bkT[:])
            nc.scalar.dma_start(biasv[:], bvT[:])
            nc.scalar.dma_start(bor[:], bo_row[:])
            nc.scalar.dma_start(onesr[:], ones_row[:])
            shiftc = cpool.tile([128, 1], F32, tag="shiftc")
            nc.vector.memset(shiftc[:], -SHIFT)

            def _load_w(pool, which, tag):
                # quarter q covers contraction tiles it=4q..4q+4, i.e. WT rows
                # 512q..512q+512 = rank 2q (i=0,1) then rank 2q+1 (i=0,1)
                parts = []
                for q in range(4):
                    wq = pool.tile([128, 4, D], F16, tag=f"{tag}{q}", name=f"{tag}{q}")
                    if which == "q":
                        nc.sync.dma_start(wq[:], Wqv[:, 4 * q:4 * q + 4, :])
                    elif which == "k":
                        nc.sync.dma_start(wq[:], AGkv[:, 4 * q:4 * q + 4, :])
                    else:
                        blk = {"v": 0, "o": 1}[which]
                        a0 = 8 * q + 2 * blk
                        nc.sync.dma_start(wq[:, 0:2, :], AGv[:, a0:a0 + 2, :])
                        nc.sync.dma_start(wq[:, 2:4, :], AGv[:, a0 + 4:a0 + 6, :])
                    parts.append(wq)
                return parts

            def _phases():
                # ---------------- Phase A: projections ----------------
                with (
                    tc.tile_pool(name="wt", bufs=1) as wpool,
                    tc.tile_pool(name="xs", bufs=2) as xpool,
                    tc.tile_pool(name="psA", bufs=8, space="PSUM") as psA,
                    tc.tile_pool(name="stA", bufs=1) as stA,
                ):
                    for xin, which, bias, spills in (
                        (qT, "q", biasq, QT_ds),
                        (kT, "k", biask, KT_ds),
                        (vT, "v", biasv, VT_ds),
                    ):
                        xs0 = xpool.tile([128, KT, TAe], F16, tag="xs", name="xs0")
                        nc.sync.dma_start(
                            xs0[:], xin[:, 0:TAe].rearrange("(it p) t -> p it t", p=128))
                        wt = _load_w(wpool, which, "wt")
                        for c in range(NCH):
                            if c == 0:
                                xs = xs0
                            else:
                                xs = xpool.tile([128, KT, TAe], F16, tag="xs")
                                nc.sync.dma_start(
                                    xs[:], xin[:, c * TAe:(c + 1) * TAe].rearrange(
                                        "(it p) t -> p it t", p=128))
                            stg = stA.tile([128, TAe, H], F16, tag="stA")
                            for jh in range(2):
                                pss = [psA.tile([128, TAe], F32, tag="psA",
                                                name=f"psA{jh}_{j}") for j in range(8)]
                                for q in range(4):
                                    for jl in range(8):
                                        jt = jh * 8 + jl
                                        for kl in range(4):
                                            ki = q * 4 + kl
                                            nc.tensor.matmul(
                                                pss[jl][:], wt[q][:, kl, jt * 128:(jt + 1) * 128],
                                                xs[:, ki, :], start=(ki == 0), stop=(ki == KT - 1))
                                for jl in range(8):
                                    jt = jh * 8 + jl
                                    nc.any.tensor_scalar_add(stg[:, :, jt], pss[jl][:],
                                                             bias[:, jt:jt + 1])
                            nc.sync.dma_start(
                                spills[c][:], stg[:].rearrange("p t h -> p (t h)"))

                # ---------------- Phase B (with Wo q0 prefetch) + C ----------------
                with tc.tile_pool(name="wo", bufs=1) as wopool:
                  wo0 = wopool.tile([128, 4, D], F16, tag="wo0", name="wo0")
                  nc.sync.dma_start(wo0[:, 0:2, :], AGv[:, 2:4, :])
                  nc.sync.dma_start(wo0[:, 2:4, :], AGv[:, 6:8, :])
                  with (
                      tc.tile_pool(name="qk", bufs=2) as qkpool,
                      tc.tile_pool(name="vb", bufs=2) as vpool,
                      tc.tile_pool(name="attc", bufs=2) as apool,
                      tc.tile_pool(name="eb", bufs=6) as epool,
                      tc.tile_pool(name="zb", bufs=8) as zpool,
                      tc.tile_pool(name="psS", bufs=2, space="PSUM") as psS,
                      tc.tile_pool(name="psT", bufs=2, space="PSUM") as psT,
                      tc.tile_pool(name="psV", bufs=2, space="PSUM") as psV,
                      tc.tile_pool(name="psA2", bufs=2, space="PSUM") as psA2,
                  ):
                      for c in range(NCH):
                          QTs = qkpool.tile([128, TAe, H], F16, tag="QTs")
                          KTs = qkpool.tile([128, TAe, H], F16, tag="KTs")
                          nc.gpsimd.dma_start(
                              QTs[:], QT_ds[c][:].rearrange("p (t h) -> p t h", h=H))
                          nc.gpsimd.dma_start(
                              KTs[:], KT_ds[c][:].rearrange("p (t h) -> p t h", h=H))
                          VTs = vpool.tile([128, TAe, H], F16, tag="VTs")
                          nc.sync.dma_start(
                              VTs[:], VT_ds[c][:].rearrange("p (t h) -> p t h", h=H))
                          ATTc = apool.tile([128, H, TAe], F16, tag="ATTc")
                          for bk in range(NBK):
                              sl = slice(bk * 8, (bk + 1) * 8)
                              w0 = (bk // 2) * 2            # even-aligned 2-block window
                              off = (bk % 2) * 128          # valid column offset
                              slw = slice(w0 * 8, (w0 + 2) * 8)
                              ps_b = psS.tile([128, 256], F32, tag="ps_s")
                              nc.tensor.matmul(
                                  ps_b[:],
                                  QTs[:, sl, :].rearrange("p t h -> p (t h)"),
                                  KTs[:, slw, :].rearrange("p t h -> p (t h)"),
                                  start=True, stop=False, skip_group_check=True)
                              nc.tensor.matmul(ps_b[:, off:off + 128], u8[:], v8[:],
                                               start=False, stop=True, skip_group_check=True)
                              E = epool.tile([128, 128], F32, tag="E")
                              Z = zpool.tile([128, 1], F32, tag="Z")
                              nc.scalar.activation(E[:], ps_b[:, off:off + 128], Exp,
                                                   bias=shiftc[:], accum_out=Z[:])
                              R = zpool.tile([128, 1], F32, tag="R")
                              nc.vector.reciprocal(R[:], Z[:])
                              Wb = epool.tile([128, 128], F16, tag="Wb")
                              nc.vector.tensor_scalar_mul(Wb[:], E[:], R[:])
                              ps_t = psT.tile([128, 128], F16, tag="ps_t")
                              nc.tensor.transpose(ps_t[:], Wb[:], ident[:])
                              WTs = epool.tile([128, 128], F16, tag="WTs")
                              nc.any.tensor_copy(WTs[:], ps_t[:])
                              ps_v = psV.tile([128, 128], F16, tag="ps_v")
                              nc.tensor.transpose(
                                  ps_v[:], VTs[:, sl, :].rearrange("p t h -> p (t h)"),
                                  ident[:])
                              Vb = epool.tile([128, 128], F16, tag="Vb")
                              nc.any.tensor_copy(Vb[:], ps_v[:])
                              ps_a = psA2.tile([128, 128], F32, tag="ps_a")
                              nc.tensor.matmul(ps_a[:], Vb[:], WTs[:],
                                               start=True, stop=True)
                              nc.any.tensor_copy(
                                  ATTc[:, :, bk * 8:(bk + 1) * 8].rearrange("p h t -> p t h"),
                                  ps_a[:].rearrange("p (t h) -> p t h", t=8))
                          nc.sync.dma_start(
                              ATT_ds[c][:].rearrange("(h p) t -> p h t", p=128), ATTc[:])

                  # ---------------- Phase C: output projection ----------------
                  with (
                      tc.tile_pool(name="ca", bufs=2) as capool,
                      tc.tile_pool(name="psC", bufs=8, space="PSUM") as psC,
                      tc.tile_pool(name="stC", bufs=4) as stC,
                  ):
                      ATTs0 = capool.tile([128, H, TAe], F16, tag="ATTs", name="ATTs0")
                      nc.sync.dma_start(
                          ATTs0[:], ATT_ds[0][:].rearrange("(h p) t -> p h t", p=128))
                      wo = [wo0]
                      for q in range(1, 4):
                          wq = wopool.tile([128, 4, D], F16, tag=f"wo{q}", name=f"wo{q}")
                          a0 = 8 * q + 2
                          nc.sync.dma_start(wq[:, 0:2, :], AGv[:, a0:a0 + 2, :])
                          nc.sync.dma_start(wq[:, 2:4, :], AGv[:, a0 + 4:a0 + 6, :])
                          wo.append(wq)
                      for cc in range(NCH):
                          if cc == 0:
                              ATTs = ATTs0
                          else:
                              ATTs = capool.tile([128, H, TAe], F16, tag="ATTs")
                              nc.sync.dma_start(
                                  ATTs[:], ATT_ds[cc][:].rearrange("(h p) t -> p h t", p=128))
                          tjs = [(tt, jc) for tt in range(TAe // 128) for jc in range(D // 512)]
                          pss = [psC.tile([128, 512], F32, tag="psC", name=f"psC{i}")
                                 for i in range(len(tjs))]
                          for hq in range(4):
                              for i, (tt, jc) in enumerate(tjs):
                                  for hl in range(4):
                                      h = hq * 4 + hl
                                      nc.tensor.matmul(
                                          pss[i][:], ATTs[:, h, tt * 128:(tt + 1) * 128],
                                          wo[hq][:, hl, jc * 512:(jc + 1) * 512],
                                          start=(h == 0), stop=False)
                          for i, (tt, jc) in enumerate(tjs):
                              nc.tensor.matmul(pss[i][:], onesr[:], bor[:, jc * 512:(jc + 1) * 512],
                                               start=False, stop=True)
                              st = stC.tile([128, 512], F16, tag="stC")
                              nc.any.tensor_copy(st[:], pss[i][:])
                              nc.sync.dma_start(
                                  out_d[cc * TAe + tt * 128: cc * TAe + (tt + 1) * 128,
                                        jc * 512:(jc + 1) * 512], st[:])

            for _rep in range(repeat):
                _phases()
    nc.compile()
    return nc


_cache = {}


def get_nc(T):
    if T not in _cache:
        _cache[T] = build(T)
    return _cache[T]


def make_in_maps(q, k, v, Wq, bq, Wk, bk, Wv, bv, Wo, bo, ncores=NCORES, T=None):
    f, h = np.float32, np.float16
    q = np.asarray(q, f).reshape(-1, D)
    k = np.asarray(k, f).reshape(-1, D)
    v = np.asarray(v, f).reshape(-1, D)
    if T is None:
        T = q.shape[0] // ncores
    WqT = np.asarray(Wq, f).T
    WkT = np.asarray(Wk, f).T
    WvT = np.asarray(Wv, f).T
    WoT = np.asarray(Wo, f).T
    bqT = np.ascontiguousarray(np.asarray(bq, f).reshape(H, 128).T)
    bkT = np.ascontiguousarray(np.asarray(bk, f).reshape(H, 128).T)
    bvTc = np.ascontiguousarray(np.asarray(bv, f).reshape(H, 128).T)
    bor = np.asarray(bo, f).reshape(1, D).astype(h)
    WqTf_h = np.ascontiguousarray(WqT).astype(h)
    maps = []
    for c in range(ncores):
        sl = slice(c * T, (c + 1) * T)
        rs = slice(c * RSH, (c + 1) * RSH)
        maps.append({
            "qT": q[sl].T.astype(h),
            "kT": k[sl].T.astype(h),
            "vT": v[sl].T.astype(h),
            "WqTf": WqTf_h,
            "Wsh": np.vstack([WkT[rs], WvT[rs], WoT[rs]]).astype(h),
            "bqT": bqT, "bkT": bkT, "bvT": bvTc, "bo_row": bor,
            "ones_row": np.ones((1, 128), h),
        })
    return maps, T


def kernel(q, k, v, Wq, bq, Wk, bk, Wv, bv, Wo, bo):
    maps, T = make_in_maps(q, k, v, Wq, bq, Wk, bk, Wv, bv, Wo, bo)
    nc = get_nc(T)
    res = run_bass_kernel_spmd(nc, maps, list(range(NCORES)))
    out = np.concatenate([np.asarray(r["out"]) for r in res.results], axis=0)
    return out.reshape(B, S, D).astype(np.float32)


# revision 51
# speedup vs baseline: 2.2015x; 1.8359x over previous
"""Trainium2 Bass kernel for nn_MultiHeadAttention_67379446939752.

Per-token multi-head attention:
  Q = q @ Wq.T + bq ; K,V likewise        [B,S,D] -> [B,S,H,HD]
  score[t,h,g] = sum_d Q[t,h,d] K[t,g,d]  (per-token HxH gram, no seq mixing)
  attn[t] = softmax(score[t]) @ V[t]      -> [B,S,D]
  out = attn @ Wo.T + bo
with B,S,D,H = 4,4096,2048,16.

Strategy: data-parallel over the 16384 tokens across 8 NeuronCores (2048
tokens/core).  Everything moves in fp16: activations arrive pre-transposed
[D,T] fp16.  Wq is replicated (8 MB/core) so the first projection starts
with zero collective wait; Wk/Wv/Wo are sharded 8-ways by contraction rows
(3 MB/core) and AllGathered on device in two steps (Wk first, then Wv|Wo)
that hide behind the Q/K projections' PE work.  Per-call host->device
traffic is ~294 MB + 67 MB out instead of ~896 MB + 128 MB (weights were
previously replicated x8 in fp32), and CoreSim-predicted exec is 1.20 ms
vs the 1.30 ms baseline (PE busy 82%, near the 0.98 ms fp32 PE roofline).
Matmuls run fp16 x fp16 -> fp32 PSUM at full PE rate.  The per-token 16x16
attention is computed 8 tokens at a time as one 128x128x128 matmul whose
cross-token blocks are pushed to -1024 in PSUM by a rank-8 mask matmul;
exp() then zeroes them exactly, so the block-diagonal softmax needs no
masking pass.  Output returns as fp16 and is upcast on host.
"""
import sys
sys.path.insert(0, "/opt/trn_rl_repo")
import numpy as np
import concourse.bass as bass
import concourse.mybir as mybir
import concourse.bacc as bacc
import concourse.tile as tile
from concourse.bass_utils import run_bass_kernel_spmd

B, S, D, H, HD = 4, 4096, 2048, 16, 128
NCORES = 8
T_FULL = B * S
F32, F16 = mybir.dt.float32, mybir.dt.float16
KT = D // 128             # contraction tiles
RSH = D // NCORES         # weight rows per rank shard (256)
SHIFT = 25.0              # constant softmax shift (softmax-invariant)
NEG = 1024.0              # additive mask magnitude for cross-token blocks
TA = 256                  # token chunk (phase A/B/C share this granularity)
Exp = mybir.ActivationFunctionType.Exp


def mask_consts():
    # u8[r,(t,h)] = 1 if t==r ; v8[r,(t',g)] = -NEG*(1 - (t'==r))
    u = np.zeros((8, 128), np.float32)
    for r in range(8):
        u[r, r * 16:(r + 1) * 16] = 1.0
    v = np.full((8, 128), -NEG, np.float32)
    for r in range(8):
        v[r, r * 16:(r + 1) * 16] = 0.0
    return u, v


def build(T, repeat=1, trace_sim=False):
    TAe = min(TA, T)
    NCH = T // TAe           # chunks
    NBK = TAe // 8           # 8-token blocks per chunk
    nc = bacc.Bacc(None, target_bir_lowering=False)
    qT = nc.dram_tensor("qT", [D, T], F16, kind="ExternalInput")
    kT = nc.dram_tensor("kT", [D, T], F16, kind="ExternalInput")
    vT = nc.dram_tensor("vT", [D, T], F16, kind="ExternalInput")
    # Wq replicated (gates phase A start: no collective wait before first matmul)
    WqTf = nc.dram_tensor("WqTf", [D, D], F16, kind="ExternalInput")
    # rank shard: rows [c*256:(c+1)*256] of WkT,WvT,WoT stacked -> [768,D]
    Wsh = nc.dram_tensor("Wsh", [3 * RSH, D], F16, kind="ExternalInput")
    bqT = nc.dram_tensor("bqT", [128, H], F32, kind="ExternalInput")
    bkT = nc.dram_tensor("bkT", [128, H], F32, kind="ExternalInput")
    bvT = nc.dram_tensor("bvT", [128, H], F32, kind="ExternalInput")
    bo_row = nc.dram_tensor("bo_row", [1, D], F16, kind="ExternalInput")
    ones_row = nc.dram_tensor("ones_row", [1, 128], F16, kind="ExternalInput")
    out_d = nc.dram_tensor("out", [T, D], F16, kind="ExternalOutput")

    u8_np, v8_np = mask_consts()
    u8_d = nc.inline_tensor(u8_np.astype(np.float16), "u8c")
    v8_d = nc.inline_tensor(v8_np.astype(np.float16), "v8c")
    id_d = nc.inline_tensor(np.eye(128, dtype=np.float16), "id128")

    with tile.TileContext(nc, trace_sim=trace_sim) as tc:
        with (
            tc.tile_pool(name="dram", bufs=1, space="DRAM") as dpool,
            tc.tile_pool(name="const", bufs=1) as cpool,
        ):
            # ---- weight AllGathers for Wk then Wv|Wo (hide behind the Q/K
            # projections; Q reads the replicated WqTf input directly)
            # bounce tiles shaped [2, N] so the DRAM->DRAM copies are two wide
            # descriptors instead of hundreds of narrow rows
            agk_in = dpool.tile([2, RSH * D // 2], F16, tag="agk_in", name="agk_in")
            agk_out = dpool.tile([NCORES * RSH, D], F16, tag="agk_out",
                                 name="agk_out", addr_space="Shared")
            ag_in = dpool.tile([2, RSH * D], F16, tag="ag_in", name="ag_in")
            ag_out = dpool.tile([NCORES * 2 * RSH, D], F16, tag="ag_out",
                                name="ag_out", addr_space="Shared")
            nc.gpsimd.dma_start(
                agk_in[:], Wsh[0:RSH, :].rearrange("(a r) c -> a (r c)", a=2))
            nc.gpsimd.collective_compute(
                "AllGather", mybir.AluOpType.bypass,
                replica_groups=[list(range(NCORES))],
                ins=[agk_in.opt()], outs=[agk_out.opt()],
            )
            nc.gpsimd.dma_start(
                ag_in[:], Wsh[RSH:3 * RSH, :].rearrange("(a r) c -> a (r c)", a=2))
            nc.gpsimd.collective_compute(
                "AllGather", mybir.AluOpType.bypass,
                replica_groups=[list(range(NCORES))],
                ins=[ag_in.opt()], outs=[ag_out.opt()],
            )
            # views [128p, a, D]:
            #  Wqv:  replicated Wq row = a*128+p                    (a in 0..16)
            #  AGkv: gathered Wk row = a*128+p, a = rank*2 + i      (a in 0..16)
            #  AGv:  gathered Wv/Wo row: a = rank*4 + blk*2 + i, blk v=0 o=1
            Wqv = WqTf.ap().rearrange("(a p) j -> p a j", p=128)
            AGkv = agk_out[:].rearrange("(a p) j -> p a j", p=128)
            AGv = ag_out[:].rearrange("(a p) j -> p a j", p=128)

            # per-chunk spill tiles (fine-grained cross-phase deps)
            QT_ds = [dpool.tile([128, TAe * H], F16, tag=f"QTd{i}", name=f"QTd{i}") for i in range(NCH)]
            KT_ds = [dpool.tile([128, TAe * H], F16, tag=f"KTd{i}", name=f"KTd{i}") for i in range(NCH)]
            VT_ds = [dpool.tile([128, TAe * H], F16, tag=f"VTd{i}", name=f"VTd{i}") for i in range(NCH)]
            ATT_ds = [dpool.tile([D, TAe], F16, tag=f"ATTd{i}", name=f"ATTd{i}") for i in range(NCH)]

            # const loads on the DVE queue: keeps the SP queue free for the
            # first xs/wt loads that gate the first matmul
            u8 = cpool.tile([8, 128], F16, tag="u8")
            v8 = cpool.tile([8, 128], F16, tag="v8")
            ident = cpool.tile([128, 128], F16, tag="ident")
            nc.scalar.dma_start(u8[:], u8_d[:])
            nc.scalar.dma_start(v8[:], v8_d[:])
            nc.scalar.dma_start(ident[:], id_d[:])
            biasq = cpool.tile([128, H], F32, tag="bq")
            biask = cpool.tile([128, H], F32, tag="bk")
            biasv = cpool.tile([128, H], F32, tag="bvt")
            bor = cpool.tile([1, D], F16, tag="bo")
            onesr = cpool.tile([1, 128], F16, tag="ones")
            nc.scalar.dma_start(biasq[:], bqT[:])
            nc.scalar.dma_start(biask[:], bkT[:])
            nc.scalar.dma_start(biasv[:], bvT[:])
            nc.scalar.dma_start(bor[:], bo_row[:])
            nc.scalar.dma_start(onesr[:], ones_row[:])
            shiftc = cpool.tile([128, 1], F32, tag="shiftc")
            nc.vector.memset(shiftc[:], -SHIFT)

            def _load_w(pool, which, tag):
                # quarter q covers contraction tiles it=4q..4q+4, i.e. WT rows
                # 512q..512q+512 = rank 2q (i=0,1) then rank 2q+1 (i=0,1)
                parts = []
                for q in range(4):
                    wq = pool.tile([128, 4, D], F16, tag=f"{tag}{q}", name=f"{tag}{q}")
                    if which == "q":
                        nc.sync.dma_start(wq[:], Wqv[:, 4 * q:4 * q + 4, :])
                    elif which == "k":
                        nc.sync.dma_start(wq[:], AGkv[:, 4 * q:4 * q + 4, :])
                    else:
                        blk = {"v": 0, "o": 1}[which]
                        a0 = 8 * q + 2 * blk
                        nc.sync.dma_start(wq[:, 0:2, :], AGv[:, a0:a0 + 2, :])
                        nc.sync.dma_start(wq[:, 2:4, :], AGv[:, a0 + 4:a0 + 6, :])
                    parts.append(wq)
                return parts

            def _phases():
                # ---------------- Phase A: projections ----------------
                with (
                    tc.tile_pool(name="wt", bufs=1) as wpool,
                    tc.tile_pool(name="xs", bufs=2) as xpool,
                    tc.tile_pool(name="psA", bufs=8, space="PSUM") as psA,
                    tc.tile_pool(name="stA", bufs=1) as stA,
                ):
                    for xin, which, bias, spills in (
                        (qT, "q", biasq, QT_ds),
                        (kT, "k", biask, KT_ds),
                        (vT, "v", biasv, VT_ds),
                    ):
                        xs0 = xpool.tile([128, KT, TAe], F16, tag="xs", name="xs0")
                        nc.sync.dma_start(
                            xs0[:], xin[:, 0:TAe].rearrange("(it p) t -> p it t", p=128))
                        wt = _load_w(wpool, which, "wt")
                        for c in range(NCH):
                            if c == 0:
                                xs = xs0
                            else:
                                xs = xpool.tile([128, KT, TAe], F16, tag="xs")
                                nc.sync.dma_start(
                                    xs[:], xin[:, c * TAe:(c + 1) * TAe].rearrange(
                                        "(it p) t -> p it t", p=128))
                            stg = stA.tile([128, TAe, H], F16, tag="stA")
                            for jh in range(2):
                                pss = [psA.tile([128, TAe], F32, tag="psA",
                                                name=f"psA{jh}_{j}") for j in range(8)]
                                for q in range(4):
                                    for jl in range(8):
                                        jt = jh * 8 + jl
                                        for kl in range(4):
                                            ki = q * 4 + kl
                                            nc.tensor.matmul(
                                                pss[jl][:], wt[q][:, kl, jt * 128:(jt + 1) * 128],
                                                xs[:, ki, :], start=(ki == 0), stop=(ki == KT - 1))
                                for jl in range(8):
                                    jt = jh * 8 + jl
                                    nc.any.tensor_scalar_add(stg[:, :, jt], pss[jl][:],
                                                             bias[:, jt:jt + 1])
                            nc.sync.dma_start(
                                spills[c][:], stg[:].rearrange("p t h -> p (t h)"))

                # ---------------- Phase B (with Wo q0 prefetch) + C ----------------
                with tc.tile_pool(name="wo", bufs=1) as wopool:
                  wo0 = wopool.tile([128, 4, D], F16, tag="wo0", name="wo0")
                  nc.sync.dma_start(wo0[:, 0:2, :], AGv[:, 2:4, :])
                  nc.sync.dma_start(wo0[:, 2:4, :], AGv[:, 6:8, :])
                  with (
                      tc.tile_pool(name="qk", bufs=2) as qkpool,
                      tc.tile_pool(name="vb", bufs=2) as vpool,
                      tc.tile_pool(name="attc", bufs=2) as apool,
                      tc.tile_pool(name="eb", bufs=6) as epool,
                      tc.tile_pool(name="zb", bufs=8) as zpool,
                      tc.tile_pool(name="psS", bufs=2, space="PSUM") as psS,
                      tc.tile_pool(name="psT", bufs=2, space="PSUM") as psT,
                      tc.tile_pool(name="psV", bufs=2, space="PSUM") as psV,
                      tc.tile_pool(name="psA2", bufs=2, space="PSUM") as psA2,
                  ):
                      for c in range(NCH):
                          QTs = qkpool.tile([128, TAe, H], F16, tag="QTs")
                          KTs = qkpool.tile([128, TAe, H], F16, tag="KTs")
                          nc.gpsimd.dma_start(
                              QTs[:], QT_ds[c][:].rearrange("p (t h) -> p t h", h=H))
                          nc.gpsimd.dma_start(
                              KTs[:], KT_ds[c][:].rearrange("p (t h) -> p t h", h=H))
                          VTs = vpool.tile([128, TAe, H], F16, tag="VTs")
                          nc.sync.dma_start(
                              VTs[:], VT_ds[c][:].rearrange("p (t h) -> p t h", h=H))
                          ATTc = apool.tile([128, H, TAe], F16, tag="ATTc")
                          for bk in range(NBK):
                              sl = slice(bk * 8, (bk + 1) * 8)
                              # fp16 runs at full PE rate at any free size, so
                              # stream only the valid 128 score columns (the
                              # 256-wide window was an fp32r-rate artifact)
                              ps_b = psS.tile([128, 128], F32, tag="ps_s")
                              nc.tensor.matmul(
                                  ps_b[:],
                                  QTs[:, sl, :].rearrange("p t h -> p (t h)"),
                                  KTs[:, sl, :].rearrange("p t h -> p (t h)"),
                                  start=True, stop=False, skip_group_check=True)
                              nc.tensor.matmul(ps_b[:], u8[:], v8[:],
                                               start=False, stop=True, skip_group_check=True)
                              E = epool.tile([128, 128], F32, tag="E")
                              Z = zpool.tile([128, 1], F32, tag="Z")
                              nc.scalar.activation(E[:], ps_b[:], Exp,
                                                   bias=shiftc[:], accum_out=Z[:])
                              R = zpool.tile([128, 1], F32, tag="R")
                              nc.vector.reciprocal(R[:], Z[:])
                              Wb = epool.tile([128, 128], F16, tag="Wb")
                              nc.vector.tensor_scalar_mul(Wb[:], E[:], R[:])
                              ps_t = psT.tile([128, 128], F16, tag="ps_t")
                              nc.tensor.transpose(ps_t[:], Wb[:], ident[:])
                              WTs = epool.tile([128, 128], F16, tag="WTs")
                              nc.any.tensor_copy(WTs[:], ps_t[:])
                              ps_v = psV.tile([128, 128], F16, tag="ps_v")
                              nc.tensor.transpose(
                                  ps_v[:], VTs[:, sl, :].rearrange("p t h -> p (t h)"),
                                  ident[:])
                              Vb = epool.tile([128, 128], F16, tag="Vb")
                              nc.any.tensor_copy(Vb[:], ps_v[:])
                              ps_a = psA2.tile([128, 128], F32, tag="ps_a")
                              nc.tensor.matmul(ps_a[:], Vb[:], WTs[:],
                                               start=True, stop=True)
                              nc.any.tensor_copy(
                                  ATTc[:, :, bk * 8:(bk + 1) * 8].rearrange("p h t -> p t h"),
                                  ps_a[:].rearrange("p (t h) -> p t h", t=8))
                          nc.sync.dma_start(
                              ATT_ds[c][:].rearrange("(h p) t -> p h t", p=128), ATTc[:])

                  # ---------------- Phase C: output projection ----------------
                  with (
                      tc.tile_pool(name="ca", bufs=2) as capool,
                      tc.tile_pool(name="psC", bufs=8, space="PSUM") as psC,
                      tc.tile_pool(name="stC", bufs=4) as stC,
                  ):
                      ATTs0 = capool.tile([128, H, TAe], F16, tag="ATTs", name="ATTs0")
                      nc.sync.dma_start(
                          ATTs0[:], ATT_ds[0][:].rearrange("(h p) t -> p h t", p=128))
                      wo = [wo0]
                      for q in range(1, 4):
                          wq = wopool.tile([128, 4, D], F16, tag=f"wo{q}", name=f"wo{q}")
                          a0 = 8 * q + 2
                          nc.sync.dma_start(wq[:, 0:2, :], AGv[:, a0:a0 + 2, :])
                          nc.sync.dma_start(wq[:, 2:4, :], AGv[:, a0 + 4:a0 + 6, :])
                          wo.append(wq)
                      for cc in range(NCH):
                          if cc == 0:
                              ATTs = ATTs0
                          else:
                              ATTs = capool.tile([128, H, TAe], F16, tag="ATTs")
                              nc.sync.dma_start(
                                  ATTs[:], ATT_ds[cc][:].rearrange("(h p) t -> p h t", p=128))
                          tjs = [(tt, jc) for tt in range(TAe // 128) for jc in range(D // 512)]
                          pss = [psC.tile([128, 512], F32, tag="psC", name=f"psC{i}")
                                 for i in range(len(tjs))]
                          for hq in range(4):
                              for i, (tt, jc) in enumerate(tjs):
                                  for hl in range(4):
                                      h = hq * 4 + hl
                                      nc.tensor.matmul(
                                          pss[i][:], ATTs[:, h, tt * 128:(tt + 1) * 128],
                                          wo[hq][:, hl, jc * 512:(jc + 1) * 512],
                                          start=(h == 0), stop=False)
                          for i, (tt, jc) in enumerate(tjs):
                              nc.tensor.matmul(pss[i][:], onesr[:], bor[:, jc * 512:(jc + 1) * 512],
                                               start=False, stop=True)
                              st = stC.tile([128, 512], F16, tag="stC")
                              nc.any.tensor_copy(st[:], pss[i][:])
                              nc.sync.dma_start(
                                  out_d[cc * TAe + tt * 128: cc * TAe + (tt + 1) * 128,
                                        jc * 512:(jc + 1) * 512], st[:])

            for _rep in range(repeat):
                _phases()
    nc.compile()
    return nc


_cache = {}


def get_nc(T):
    if T not in _cache:
        _cache[T] = build(T)
    return _cache[T]


def make_in_maps(q, k, v, Wq, bq, Wk, bk, Wv, bv, Wo, bo, ncores=NCORES, T=None):
    f, h = np.float32, np.float16
    q = np.asarray(q, f).reshape(-1, D)
    k = np.asarray(k, f).reshape(-1, D)
    v = np.asarray(v, f).reshape(-1, D)
    if T is None:
        T = q.shape[0] // ncores
    WqT = np.asarray(Wq, f).T
    WkT = np.asarray(Wk, f).T
    WvT = np.asarray(Wv, f).T
    WoT = np.asarray(Wo, f).T
    bqT = np.ascontiguousarray(np.asarray(bq, f).reshape(H, 128).T)
    bkT = np.ascontiguousarray(np.asarray(bk, f).reshape(H, 128).T)
    bvTc = np.ascontiguousarray(np.asarray(bv, f).reshape(H, 128).T)
    bor = np.asarray(bo, f).reshape(1, D).astype(h)
    WqTf_h = np.ascontiguousarray(WqT).astype(h)
    maps = []
    for c in range(ncores):
        sl = slice(c * T, (c + 1) * T)
        rs = slice(c * RSH, (c + 1) * RSH)
        maps.append({
            "qT": q[sl].T.astype(h),
            "kT": k[sl].T.astype(h),
            "vT": v[sl].T.astype(h),
            "WqTf": WqTf_h,
            "Wsh": np.vstack([WkT[rs], WvT[rs], WoT[rs]]).astype(h),
            "bqT": bqT, "bkT": bkT, "bvT": bvTc, "bo_row": bor,
            "ones_row": np.ones((1, 128), h),
        })
    return maps, T


def kernel(q, k, v, Wq, bq, Wk, bk, Wv, bv, Wo, bo):
    maps, T = make_in_maps(q, k, v, Wq, bq, Wk, bk, Wv, bv, Wo, bo)
    nc = get_nc(T)
    res = run_bass_kernel_spmd(nc, maps, list(range(NCORES)))
    out = np.concatenate([np.asarray(r["out"]) for r in res.results], axis=0)
    return out.reshape(B, S, D).astype(np.float32)
